# revision 9
# baseline (speedup 1.0000x reference)
"""Trainium2 Bass kernel for ContinuousREWAEncoder:
    out = FWHT(x @ W^T)/sqrt(32) + 0.01*normal(key=42)

Math folding: FWHT is linear => out = x @ (H @ W / sqrt(32))^T + noise.
The noise uses a fixed PRNG key => deterministic constant, added on HOST
(zero device cost, bit-identical to the reference noise).

Device math (per core, data parallel over tokens):
  x is streamed as fp8e4m3 (hi, lo) pairs:   x ~= xhi + xlo/16
  w is held as fp8 cells in a [128, 2, 64] DoubleRow stationary:
     out rows  0:32 cells (whi,    whi/16 ) -> psumA = whi*x
     out rows 32:64 cells (wlo/16, wlo/256) -> psumB = (wlo/16)*x
  where 16*w_eff ~= whi + wlo/16.  A DoubleRow matmul ingests both fp8
  planes in one pass, and psumA+psumB = 16*w_eff*x to ~1e-3 max rel err.
  DVE (+Act for the tail block) stages psumB into SBUF and adds psumA
  -> fp16; the host divides by 16 and adds the noise.

DMA strategy: descriptors are assigned to the 16 DMA engines round-robin
by descriptor index, restarting at engine 0 for every queue entry -- and
engine 15 (the queue-management engine) runs ~20% slower than the rest,
so a 128-run entry always straggles on its last 8 partitions.  Every x
segment is therefore issued as NINE sub-entries of <=15 partition runs:
engine 15 receives nothing and the other fifteen engines stream at full
rate.  Segments keep >=2 KiB contiguous per-partition runs, the middle
of the stream uses 16 KiB paired-block runs, and the last block arrives
in chunk pieces (4,2,2) so only two matmuls trail the final byte.
"""

import math

import ml_dtypes
import numpy as np

import concourse.tile as tile
from concourse import bacc, mybir
from concourse.bass_utils import run_bass_kernel_spmd

B, N, D, M = 4, 8192, 1024, 32
NOISE_STD = 0.01
N_CORES = 8
TOK_TOTAL = B * N              # 32768
TOK = TOK_TOTAL // N_CORES     # 4096 tokens per core
BLK = 512                      # tokens per PSUM bank ([64, 512] fp32 = 1 bank)
NBLK = TOK // BLK              # 8
KC = D // 128                  # 8 contraction chunks of 128 dims

FP8 = mybir.dt.float8e4
NP8 = ml_dtypes.float8_e4m3    # == mybir.dt.np(mybir.dt.float8e4)
F32 = mybir.dt.float32
F16 = mybir.dt.float16
DR = mybir.MatmulPerfMode.DoubleRow

X_BYTES = TOK * D * 2 // 128   # 65536 fp8 bytes per partition per core
LAST_PIECES = (4, 2, 2)        # chunk split of the final 512 block
HALF = BLK // 2

# partition sub-ranges per DMA entry: 8x15 + 8 -> engine 15 never used
PRANGES = [(i * 15, 15) for i in range(8)] + [(120, 8)]


def _build_bass():
    nc = bacc.Bacc("TRN2", target_bir_lowering=False)

    # per-partition byte stream, every segment contiguous [blk][c][i][t]:
    # [b0 b1 | b2 b3 | b4 b5 | b6 | b7c0-3 | b7c4-5 | b7c6-7]
    xT = nc.dram_tensor("xT", [128, X_BYTES], FP8, kind="ExternalInput")
    wT = nc.dram_tensor("wT", [128, KC * 2 * 64], FP8, kind="ExternalInput")
    outT = nc.dram_tensor("outT", [M, TOK], F16, kind="ExternalOutput")

    with tile.TileContext(nc) as tc:
        with (
            tc.tile_pool(name="w", bufs=1) as wpool,
            tc.tile_pool(name="x", bufs=1) as xpool,
            tc.tile_pool(name="out", bufs=1) as opool,
            tc.tile_pool(name="sb", bufs=1) as spool,
            tc.tile_pool(name="psum", bufs=NBLK, space="PSUM") as ppool,
        ):
            # w on the scalar ring, ahead of the out DMAs; the sync ring
            # carries only the x stream.
            w_tile = wpool.tile([128, KC, 2, 64], FP8)
            nc.scalar.dma_start(
                w_tile[:], wT.rearrange("p (c i m) -> p c i m", c=KC, i=2)
            )

            off = 0

            def fetch(nbytes, tag):
                nonlocal off
                t = xpool.tile([128, nbytes], FP8, tag=tag)
                for p0, pn in PRANGES:
                    nc.sync.dma_start(
                        t[p0 : p0 + pn, :], xT[p0 : p0 + pn, off : off + nbytes]
                    )
                off += nbytes
                return t

            rhs_of = {}
            for g, pair in enumerate(((0, 1), (2, 3), (4, 5))):  # 16 KiB runs
                t = fetch(2 * KC * 2 * BLK, f"xg{g}")
                v = t.rearrange("p (b c i t) -> p b c i t", b=2, c=KC, i=2)
                for half in range(2):
                    rhs_of[pair[half]] = lambda c, v=v, half=half: v[:, half, c]
            t6 = fetch(KC * 2 * BLK, "x6")
            v6 = t6.rearrange("p (c i t) -> p c i t", c=KC, i=2)
            rhs_of[6] = lambda c: v6[:, c]

            piece_views = []
            c0 = 0
            for pi, npc in enumerate(LAST_PIECES):
                tp = fetch(npc * 2 * BLK, f"xp{pi}")
                vp = tp.rearrange("p (c i t) -> p c i t", c=npc, i=2)
                piece_views.append((c0, npc, vp))
                c0 += npc
            rhs_of[7] = lambda c: next(
                vp[:, c - pc0] for pc0, npc, vp in piece_views if pc0 <= c < pc0 + npc
            )

            # The matmul codegen supports a single sync wait; this warmup
            # matmul absorbs the w-DMA wait into PE program order so every
            # real matmul needs only its x-DMA wait.
            warm = ppool.tile([64, 64], F32, tag="ptile")
            nc.tensor.matmul(warm[:], w_tile[:, 0], w_tile[:, 0], perf_mode=DR)

            for b in range(NBLK):
                ptile = ppool.tile([64, BLK], F32, tag="ptile")
                for c in range(KC):
                    nc.tensor.matmul(
                        ptile[:],
                        w_tile[:, c],
                        rhs_of[b](c),
                        start=(c == 0),
                        stop=(c == KC - 1),
                        perf_mode=DR,
                    )

                o_tile = opool.tile([M, BLK], F16, tag=f"o{b}")
                if b < NBLK - 1:
                    # psumA+psumB -> fp16 on DVE alone (copy then add).
                    sB = spool.tile([M, BLK], F32, tag=f"s{b}")
                    nc.vector.tensor_copy(sB[:], ptile[M : 2 * M, :])
                    nc.vector.tensor_add(o_tile[:], ptile[0:M, :], sB[:])
                else:
                    # tail block: Act copies one half while DVE copies the
                    # other, then DVE runs the two adds back to back.
                    sB = spool.tile([M, BLK], F32, tag=f"s{b}")
                    nc.vector.tensor_copy(sB[:, 0:HALF], ptile[M : 2 * M, 0:HALF])
                    nc.scalar.copy(sB[:, HALF:BLK], ptile[M : 2 * M, HALF:BLK])
                    nc.vector.tensor_add(
                        o_tile[:, 0:HALF], ptile[0:M, 0:HALF], sB[:, 0:HALF]
                    )
                    nc.vector.tensor_add(
                        o_tile[:, HALF:BLK], ptile[0:M, HALF:BLK], sB[:, HALF:BLK]
                    )
                nc.scalar.dma_start(outT[:, b * BLK : (b + 1) * BLK], o_tile[:])

    nc.compile()
    return nc


_NC_CACHE = None


def _get_nc():
    global _NC_CACHE
    if _NC_CACHE is None:
        _NC_CACHE = _build_bass()
    return _NC_CACHE


def _hadamard32() -> np.ndarray:
    h = np.array([[1.0]], dtype=np.float64)
    while h.shape[0] < M:
        h = np.block([[h, h], [h, -h]])
    return h


_NOISE_CACHE = None


def _noise() -> np.ndarray:
    # Mirror reference.py exactly (same op on the default jax backend).
    global _NOISE_CACHE
    if _NOISE_CACHE is None:
        import jax

        nz = NOISE_STD * jax.random.normal(
            jax.random.key(42), (B, N, M), dtype=np.float32
        )
        _NOISE_CACHE = np.asarray(nz)
    return _NOISE_CACHE


def _pack_w(W: np.ndarray) -> np.ndarray:
    """Build the DoubleRow stationary cells [128, KC*2*64] fp8."""
    w_eff = (_hadamard32() @ W.astype(np.float64)) / math.sqrt(M)  # [M, D]
    W16 = 16.0 * w_eff
    whi = W16.astype(np.float32).astype(NP8)
    wlo = (16.0 * (W16 - whi.astype(np.float64))).astype(np.float32).astype(NP8)
    whi_f = whi.astype(np.float32)
    wlo_f = wlo.astype(np.float32)

    cells = np.empty((2, 64, D), dtype=NP8)  # [i, m, d]
    cells[0, 0:M] = whi                       # pairs with xhi
    cells[0, M:] = (wlo_f / 16.0).astype(NP8)
    cells[1, 0:M] = (whi_f / 16.0).astype(NP8)  # pairs with xlo (=16*residual)
    cells[1, M:] = (wlo_f / 256.0).astype(NP8)

    # [i, m, c, p] -> [p, c, i, m]
    wf = cells.reshape(2, 64, KC, 128).transpose(3, 2, 0, 1)
    return np.ascontiguousarray(wf).reshape(128, KC * 2 * 64)


def _pack_x_core(xhi: np.ndarray, xlo: np.ndarray) -> np.ndarray:
    """[TOK, D] hi/lo fp8 -> [128, X_BYTES] per-partition stream."""

    def seg(t0, tn, c0=0, cn=KC):
        q = np.stack([xhi[t0 : t0 + tn], xlo[t0 : t0 + tn]])  # [2, n, D]
        qr = q.reshape(2, tn, KC, 128)[:, :, c0 : c0 + cn]    # [2, n, cn, 128]
        arr = qr.transpose(3, 2, 0, 1)                        # [128, cn, 2, n]
        return arr.reshape(128, cn * 2 * tn)

    segs = [seg(b * BLK, BLK) for b in range(7)]
    c0 = 0
    for npc in LAST_PIECES:
        segs.append(seg(7 * BLK, BLK, c0, npc))
        c0 += npc
    return np.ascontiguousarray(np.concatenate(segs, axis=1))


def kernel(x: np.ndarray, W: np.ndarray, _profile_sink=None) -> np.ndarray:
    x = np.ascontiguousarray(np.asarray(x, dtype=np.float32))
    W = np.asarray(W, dtype=np.float32)

    w_dev = _pack_w(W)

    X = x.reshape(TOK_TOTAL, D)
    xhi = X.astype(NP8)
    xlo = (16.0 * (X - xhi.astype(np.float32))).astype(NP8)

    in_maps = []
    for i in range(N_CORES):
        sl = slice(i * TOK, (i + 1) * TOK)
        in_maps.append({"xT": _pack_x_core(xhi[sl], xlo[sl]), "wT": w_dev})

    res = run_bass_kernel_spmd(
        _get_nc(),
        in_maps,
        core_ids=list(range(N_CORES)),
        trace=_profile_sink is not None,
    )
    if _profile_sink is not None:
        _profile_sink.append(res)

    # device result is 16*(x @ w_eff^T), transposed, fp16
    out = np.concatenate(
        [r["outT"].T.astype(np.float32) for r in res.results], axis=0
    )
    out = out.reshape(B, N, M) * (1.0 / 16.0) + _noise()
    return np.ascontiguousarray(out.astype(np.float32))


if __name__ == "__main__":
    xs = np.random.randn(B, N, D).astype(np.float32)
    Ws = (np.random.randn(M, D) / math.sqrt(D)).astype(np.float32)
    o = kernel(xs, Ws)
    print(o.shape, o.dtype)


# revision 16
# speedup vs baseline: 1.9402x; 1.9402x over previous
"""Trainium2 Bass kernel for ContinuousREWAEncoder:
    out = FWHT(x @ W^T)/sqrt(32) + 0.01*normal(key=42)

Math folding: FWHT is linear => out = x @ (H @ W / sqrt(32))^T + noise.
The noise uses a fixed PRNG key => deterministic constant, added on HOST
(zero device cost, bit-identical to the reference noise).

Device math (per core, data parallel over tokens):
  x is streamed as fp8e4m3 (hi, lo) pairs:   x ~= xhi + xlo/16
  w is held as fp8 cells in a [128, 2, 64] DoubleRow stationary:
     out rows  0:32 cells (whi,    whi/16 ) -> psumA = whi*x
     out rows 32:64 cells (wlo/16, wlo/256) -> psumB = (wlo/16)*x
  where 16*w_eff ~= whi + wlo/16.  A DoubleRow matmul ingests both fp8
  planes in one pass, and psumA+psumB = 16*w_eff*x to ~1e-3 max rel err.
  DVE (+Act on the tail block) stages psumB into SBUF and adds psumA
  -> fp16; the host divides by 16 and adds the noise.

DMA strategy (from trace archaeology): descriptors of an entry go to the
16 DMA engines round-robin restarting at engine 0, engine 15 (which also
runs queue management) is ~20% slower than the rest, and contiguous
DRAM regions coalesce into a single descriptor.  So the host lays every
segment out as 15 groups of 8 partitions (each group contiguous, pads
in between) plus 8 single-partition runs: one grouped entry -> 15 fat
descriptors on engines 0-14, one singles entry -> engines 0-7, and the
slow engine 15 carries nothing.  Two entries per segment keeps the
per-entry queue-management cost (~1us each, serialized on engine 15)
off the critical path.  The last 512-token block arrives in chunk
pieces (4,2,2) so only two matmuls trail the final byte.
"""

import math

import ml_dtypes
import numpy as np

import concourse.tile as tile
from concourse import bacc, mybir
from concourse.bass_utils import run_bass_kernel_spmd

B, N, D, M = 4, 8192, 1024, 32
NOISE_STD = 0.01
N_CORES = 8
TOK_TOTAL = B * N              # 32768
TOK = TOK_TOTAL // N_CORES     # 4096 tokens per core
BLK = 512                      # tokens per PSUM bank ([64, 512] fp32 = 1 bank)
NBLK = TOK // BLK              # 8
KC = D // 128                  # 8 contraction chunks of 128 dims

FP8 = mybir.dt.float8e4
NP8 = ml_dtypes.float8_e4m3    # == mybir.dt.np(mybir.dt.float8e4)
F32 = mybir.dt.float32
F16 = mybir.dt.float16
DR = mybir.MatmulPerfMode.DoubleRow

LAST_PIECES = (4, 2, 2)        # chunk split of the final 512 block
HALF = BLK // 2
PAD = 512                      # DRAM pad that breaks descriptor coalescing

# segment list: (blocks, chunk-range) -> per-partition run bytes
SEGS = (
    (2, KC), (2, KC), (2, KC),  # b0b1 | b2b3 | b4b5   (16 KiB runs)
    (1, KC),                    # b6                    (8 KiB runs)
    (1, 4), (1, 2), (1, 2),     # b7 chunk pieces       (4/2/2 KiB runs)
)


def _seg_run(nblk, cn):
    return nblk * cn * 2 * BLK


S_BYTES = sum(_seg_run(nb, cn) for nb, cn in SEGS)   # 65536 per partition
G_BYTES = 8 * S_BYTES                                # 8 partitions per group


def _build_bass():
    nc = bacc.Bacc("TRN2", target_bir_lowering=False)

    # grouped plane: row g holds partitions 8g..8g+7 (contiguous per
    # segment -> one fat descriptor per row per entry); singles plane:
    # row j holds partition 120+j (8 descriptors -> engines 0-7).
    xTg = nc.dram_tensor("xTg", [15, G_BYTES], FP8, kind="ExternalInput")
    xTs = nc.dram_tensor("xTs", [8, S_BYTES], FP8, kind="ExternalInput")
    wT = nc.dram_tensor("wT", [128, KC * 2 * 64], FP8, kind="ExternalInput")
    outT = nc.dram_tensor("outT", [M, TOK], F16, kind="ExternalOutput")

    with tile.TileContext(nc) as tc:
        with (
            tc.tile_pool(name="w", bufs=1) as wpool,
            tc.tile_pool(name="x", bufs=1) as xpool,
            tc.tile_pool(name="out", bufs=1) as opool,
            tc.tile_pool(name="sb", bufs=1) as spool,
            tc.tile_pool(name="psum", bufs=NBLK, space="PSUM") as ppool,
        ):
            # w on the scalar ring, ahead of the out DMAs; the sync ring
            # carries only the x stream.
            w_tile = wpool.tile([128, KC, 2, 64], FP8)
            nc.scalar.dma_start(
                w_tile[:], wT.rearrange("p (c i m) -> p c i m", c=KC, i=2)
            )

            off = 0

            def fetch(run, tag):
                nonlocal off
                t = xpool.tile([128, run], FP8, tag=tag)
                # grouped entry: 15 descriptors x (8 partitions x run)
                nc.sync.dma_start(t[0:120], xTg[:, 8 * off : 8 * (off + run)])
                # singles entry: 8 descriptors, partitions 120-127
                nc.sync.dma_start(t[120:128], xTs[:, off : off + run])
                off += run
                return t

            rhs_of = {}
            for g, pair in enumerate(((0, 1), (2, 3), (4, 5))):
                t = fetch(_seg_run(2, KC), f"xg{g}")
                v = t.rearrange("p (b c i t) -> p b c i t", b=2, c=KC, i=2)
                for half in range(2):
                    rhs_of[pair[half]] = lambda c, v=v, half=half: v[:, half, c]
            t6 = fetch(_seg_run(1, KC), "x6")
            v6 = t6.rearrange("p (c i t) -> p c i t", c=KC, i=2)
            rhs_of[6] = lambda c: v6[:, c]

            piece_views = []
            c0 = 0
            for pi, npc in enumerate(LAST_PIECES):
                tp = fetch(_seg_run(1, npc), f"xp{pi}")
                vp = tp.rearrange("p (c i t) -> p c i t", c=npc, i=2)
                piece_views.append((c0, npc, vp))
                c0 += npc
            rhs_of[7] = lambda c: next(
                vp[:, c - pc0] for pc0, npc, vp in piece_views if pc0 <= c < pc0 + npc
            )

            # The matmul codegen supports a single sync wait; this warmup
            # matmul absorbs the w-DMA wait into PE program order so every
            # real matmul needs only its x-DMA wait.
            warm = ppool.tile([64, 64], F32, tag="ptile")
            nc.tensor.matmul(warm[:], w_tile[:, 0], w_tile[:, 0], perf_mode=DR)

            for b in range(NBLK):
                ptile = ppool.tile([64, BLK], F32, tag="ptile")
                for c in range(KC):
                    nc.tensor.matmul(
                        ptile[:],
                        w_tile[:, c],
                        rhs_of[b](c),
                        start=(c == 0),
                        stop=(c == KC - 1),
                        perf_mode=DR,
                    )

                o_tile = opool.tile([M, BLK], F16, tag=f"o{b}")
                sB = spool.tile([M, BLK], F32, tag=f"s{b}")
                if b < NBLK - 1:
                    # psumA+psumB -> fp16 on DVE alone (copy then add).
                    nc.vector.tensor_copy(sB[:], ptile[M : 2 * M, :])
                    nc.vector.tensor_add(o_tile[:], ptile[0:M, :], sB[:])
                else:
                    # tail block: Act copies one half while DVE copies the
                    # other, then DVE runs the two adds back to back.
                    nc.vector.tensor_copy(sB[:, 0:HALF], ptile[M : 2 * M, 0:HALF])
                    nc.scalar.copy(sB[:, HALF:BLK], ptile[M : 2 * M, HALF:BLK])
                    nc.vector.tensor_add(
                        o_tile[:, 0:HALF], ptile[0:M, 0:HALF], sB[:, 0:HALF]
                    )
                    nc.vector.tensor_add(
                        o_tile[:, HALF:BLK], ptile[0:M, HALF:BLK], sB[:, HALF:BLK]
                    )
                nc.scalar.dma_start(outT[:, b * BLK : (b + 1) * BLK], o_tile[:])

    nc.compile()
    return nc


_NC_CACHE = None


def _get_nc():
    global _NC_CACHE
    if _NC_CACHE is None:
        _NC_CACHE = _build_bass()
    return _NC_CACHE


def _hadamard32() -> np.ndarray:
    h = np.array([[1.0]], dtype=np.float64)
    while h.shape[0] < M:
        h = np.block([[h, h], [h, -h]])
    return h


_NOISE_CACHE = None


def _noise() -> np.ndarray:
    # Mirror reference.py exactly (same op on the default jax backend).
    global _NOISE_CACHE
    if _NOISE_CACHE is None:
        import jax

        nz = NOISE_STD * jax.random.normal(
            jax.random.key(42), (B, N, M), dtype=np.float32
        )
        _NOISE_CACHE = np.asarray(nz)
    return _NOISE_CACHE


def _pack_w(W: np.ndarray) -> np.ndarray:
    """Build the DoubleRow stationary cells [128, KC*2*64] fp8."""
    w_eff = (_hadamard32() @ W.astype(np.float64)) / math.sqrt(M)  # [M, D]
    W16 = 16.0 * w_eff
    whi = W16.astype(np.float32).astype(NP8)
    wlo = (16.0 * (W16 - whi.astype(np.float64))).astype(np.float32).astype(NP8)
    whi_f = whi.astype(np.float32)
    wlo_f = wlo.astype(np.float32)

    cells = np.empty((2, 64, D), dtype=NP8)  # [i, m, d]
    cells[0, 0:M] = whi                       # pairs with xhi
    cells[0, M:] = (wlo_f / 16.0).astype(NP8)
    cells[1, 0:M] = (whi_f / 16.0).astype(NP8)  # pairs with xlo (=16*residual)
    cells[1, M:] = (wlo_f / 256.0).astype(NP8)

    # [i, m, c, p] -> [p, c, i, m]
    wf = cells.reshape(2, 64, KC, 128).transpose(3, 2, 0, 1)
    return np.ascontiguousarray(wf).reshape(128, KC * 2 * 64)


def _pack_x_core(xhi: np.ndarray, xlo: np.ndarray):
    """[TOK, D] hi/lo fp8 -> (xTg [15, G_BYTES], xTs [8, S_BYTES])."""
    bg = np.empty((15, G_BYTES), dtype=np.uint8)
    bs = np.empty((8, S_BYTES), dtype=np.uint8)

    def seg_arr(t0, tn, c0, cn):
        q = np.stack([xhi[t0 : t0 + tn], xlo[t0 : t0 + tn]])  # [2, n, D]
        qr = q.reshape(2, tn, KC, 128)[:, :, c0 : c0 + cn]    # [2, n, cn, 128]
        arr = qr.transpose(3, 2, 0, 1)                        # [128, cn, 2, n]
        return np.ascontiguousarray(arr).view(np.uint8).reshape(128, cn * 2 * tn)

    blocks = [seg_arr(b * BLK, BLK, 0, KC) for b in range(7)]
    c0 = 0
    pieces = []
    for npc in LAST_PIECES:
        pieces.append(seg_arr(7 * BLK, BLK, c0, npc))
        c0 += npc

    # segment = what one grouped+singles entry pair fetches; a paired
    # segment's per-partition bytes are [block0 | block1]
    seg_arrs = [
        np.concatenate([blocks[0], blocks[1]], axis=1),
        np.concatenate([blocks[2], blocks[3]], axis=1),
        np.concatenate([blocks[4], blocks[5]], axis=1),
        blocks[6],
        *pieces,
    ]

    off = 0
    for a in seg_arrs:
        run = a.shape[1]
        bg[:, 8 * off : 8 * (off + run)] = a[0:120].reshape(15, 8 * run)
        bs[:, off : off + run] = a[120:128]
        off += run
    return bg.view(NP8), bs.view(NP8)


def kernel(x: np.ndarray, W: np.ndarray, _profile_sink=None) -> np.ndarray:
    x = np.ascontiguousarray(np.asarray(x, dtype=np.float32))
    W = np.asarray(W, dtype=np.float32)

    w_dev = _pack_w(W)

    X = x.reshape(TOK_TOTAL, D)
    xhi = X.astype(NP8)
    xlo = (16.0 * (X - xhi.astype(np.float32))).astype(NP8)

    in_maps = []
    for i in range(N_CORES):
        sl = slice(i * TOK, (i + 1) * TOK)
        bg, bs = _pack_x_core(xhi[sl], xlo[sl])
        in_maps.append({"xTg": bg, "xTs": bs, "wT": w_dev})

    res = run_bass_kernel_spmd(
        _get_nc(),
        in_maps,
        core_ids=list(range(N_CORES)),
        trace=_profile_sink is not None,
    )
    if _profile_sink is not None:
        _profile_sink.append(res)

    # device result is 16*(x @ w_eff^T), transposed, fp16
    out = np.concatenate(
        [r["outT"].T.astype(np.float32) for r in res.results], axis=0
    )
    out = out.reshape(B, N, M) * (1.0 / 16.0) + _noise()
    return np.ascontiguousarray(out.astype(np.float32))


if __name__ == "__main__":
    xs = np.random.randn(B, N, D).astype(np.float32)
    Ws = (np.random.randn(M, D) / math.sqrt(D)).astype(np.float32)
    o = kernel(xs, Ws)
    print(o.shape, o.dtype)


# revision 18
# speedup vs baseline: 2.9934x; 1.5428x over previous
"""Trainium2 Bass kernel for ContinuousREWAEncoder:
    out = FWHT(x @ W^T)/sqrt(32) + 0.01*normal(key=42)

Math folding: FWHT is linear => out = x @ (H @ W / sqrt(32))^T + noise.
The noise uses a fixed PRNG key => deterministic constant, added on HOST
(zero device cost, bit-identical to the reference noise).

Device math (per core, data parallel over tokens):
  x is streamed as fp8e4m3 (hi, lo) pairs:   x ~= xhi + xlo/16
  w is held as fp8 cells in a [128, 2, 64] DoubleRow stationary:
     out rows  0:32 cells (whi,    whi/16 ) -> psumA = whi*x
     out rows 32:64 cells (wlo/16, wlo/256) -> psumB = (wlo/16)*x
  where 16*w_eff ~= whi + wlo/16.  A DoubleRow matmul ingests both fp8
  planes in one pass, and psumA+psumB = 16*w_eff*x to ~1e-3 max rel err.
  The Act engine stages psumB into SBUF, DVE adds psumA -> fp16, and the
  host divides by 16 and adds the noise.

DMA structure (from trace archaeology): the per-core DMA queues are all
managed by the last of the 16 DMA engines, which therefore runs ~20%
slower than the rest and straggles the stream tail; every queue entry
also costs it ~1us of management.  So the x stream uses only SIX plain
[128 x run] entries on the sync ring (16 KiB paired-block runs, then an
8 KiB block, then the last block as 6-chunk + 2-chunk pieces so just two
matmuls trail the final byte), and the out DMAs ride the DVE ring
(managed by a different engine, and issued back-to-back with the final
add without a cross-engine hop).
"""

import math

import ml_dtypes
import numpy as np

import concourse.tile as tile
from concourse import bacc, mybir
from concourse.bass_utils import run_bass_kernel_spmd

B, N, D, M = 4, 8192, 1024, 32
NOISE_STD = 0.01
N_CORES = 8
TOK_TOTAL = B * N              # 32768
TOK = TOK_TOTAL // N_CORES     # 4096 tokens per core
BLK = 512                      # tokens per PSUM bank ([64, 512] fp32 = 1 bank)
NBLK = TOK // BLK              # 8
KC = D // 128                  # 8 contraction chunks of 128 dims

FP8 = mybir.dt.float8e4
NP8 = ml_dtypes.float8_e4m3    # == mybir.dt.np(mybir.dt.float8e4)
F32 = mybir.dt.float32
F16 = mybir.dt.float16
DR = mybir.MatmulPerfMode.DoubleRow

X_BYTES = TOK * D * 2 // 128   # 65536 fp8 bytes per partition per core
LAST_PIECES = (6, 2)           # chunk split of the final 512 block
HALF = BLK // 2


def _build_bass():
    nc = bacc.Bacc("TRN2", target_bir_lowering=False)

    # per-partition byte stream, every segment contiguous [blk][c][i][t]:
    # [b0 b1 | b2 b3 | b4 b5 | b6 | b7 c0-5 | b7 c6-7]
    xT = nc.dram_tensor("xT", [128, X_BYTES], FP8, kind="ExternalInput")
    wT = nc.dram_tensor("wT", [128, KC * 2 * 64], FP8, kind="ExternalInput")
    outT = nc.dram_tensor("outT", [M, TOK], F16, kind="ExternalOutput")

    with tile.TileContext(nc) as tc:
        with (
            tc.tile_pool(name="w", bufs=1) as wpool,
            tc.tile_pool(name="x", bufs=1) as xpool,
            tc.tile_pool(name="out", bufs=1) as opool,
            tc.tile_pool(name="sb", bufs=1) as spool,
            tc.tile_pool(name="psum", bufs=NBLK, space="PSUM") as ppool,
        ):
            # w on the scalar ring; the sync ring carries only x.
            w_tile = wpool.tile([128, KC, 2, 64], FP8)
            nc.scalar.dma_start(
                w_tile[:], wT.rearrange("p (c i m) -> p c i m", c=KC, i=2)
            )

            off = 0

            def fetch(run, tag):
                nonlocal off
                t = xpool.tile([128, run], FP8, tag=tag)
                nc.sync.dma_start(t[:], xT[:, off : off + run])
                off += run
                return t

            rhs_of = {}
            for g, pair in enumerate(((0, 1), (2, 3), (4, 5))):  # 16 KiB runs
                t = fetch(2 * KC * 2 * BLK, f"xg{g}")
                v = t.rearrange("p (b c i t) -> p b c i t", b=2, c=KC, i=2)
                for half in range(2):
                    rhs_of[pair[half]] = lambda c, v=v, half=half: v[:, half, c]
            t6 = fetch(KC * 2 * BLK, "x6")
            v6 = t6.rearrange("p (c i t) -> p c i t", c=KC, i=2)
            rhs_of[6] = lambda c: v6[:, c]

            piece_views = []
            c0 = 0
            for pi, npc in enumerate(LAST_PIECES):
                tp = fetch(npc * 2 * BLK, f"xp{pi}")
                vp = tp.rearrange("p (c i t) -> p c i t", c=npc, i=2)
                piece_views.append((c0, npc, vp))
                c0 += npc
            rhs_of[7] = lambda c: next(
                vp[:, c - pc0] for pc0, npc, vp in piece_views if pc0 <= c < pc0 + npc
            )

            # The matmul codegen supports a single sync wait; this warmup
            # matmul absorbs the w-DMA wait into PE program order so every
            # real matmul needs only its x-DMA wait.
            warm = ppool.tile([64, 64], F32, tag="ptile")
            nc.tensor.matmul(warm[:], w_tile[:, 0], w_tile[:, 0], perf_mode=DR)

            for b in range(NBLK):
                ptile = ppool.tile([64, BLK], F32, tag="ptile")
                for c in range(KC):
                    nc.tensor.matmul(
                        ptile[:],
                        w_tile[:, c],
                        rhs_of[b](c),
                        start=(c == 0),
                        stop=(c == KC - 1),
                        perf_mode=DR,
                    )

                o_tile = opool.tile([M, BLK], F16, tag=f"o{b}")
                sB = spool.tile([M, BLK], F32, tag=f"s{b}")
                if b < NBLK - 1:
                    # Act stages psumB (only DVE/Act read PSUM, one PSUM
                    # operand per op), DVE adds psumA and casts to fp16.
                    nc.scalar.copy(sB[:], ptile[M : 2 * M, :])
                    nc.vector.tensor_add(o_tile[:], ptile[0:M, :], sB[:])
                else:
                    # tail block: Act and DVE copy one half each, then DVE
                    # runs the two adds and issues the out DMA itself.
                    nc.vector.tensor_copy(sB[:, 0:HALF], ptile[M : 2 * M, 0:HALF])
                    nc.scalar.copy(sB[:, HALF:BLK], ptile[M : 2 * M, HALF:BLK])
                    nc.vector.tensor_add(
                        o_tile[:, 0:HALF], ptile[0:M, 0:HALF], sB[:, 0:HALF]
                    )
                    nc.vector.tensor_add(
                        o_tile[:, HALF:BLK], ptile[0:M, HALF:BLK], sB[:, HALF:BLK]
                    )
                # out DMAs on the gpsimd ring (SWDGE): keeps the scalar
                # engine free for psumB copies and the queue-entry
                # management off the x/w rings.
                nc.gpsimd.dma_start(outT[:, b * BLK : (b + 1) * BLK], o_tile[:])

    nc.compile()
    return nc


_NC_CACHE = None


def _get_nc():
    global _NC_CACHE
    if _NC_CACHE is None:
        _NC_CACHE = _build_bass()
    return _NC_CACHE


def _hadamard32() -> np.ndarray:
    h = np.array([[1.0]], dtype=np.float64)
    while h.shape[0] < M:
        h = np.block([[h, h], [h, -h]])
    return h


_NOISE_CACHE = None


def _noise() -> np.ndarray:
    # Mirror reference.py exactly (same op on the default jax backend).
    global _NOISE_CACHE
    if _NOISE_CACHE is None:
        import jax

        nz = NOISE_STD * jax.random.normal(
            jax.random.key(42), (B, N, M), dtype=np.float32
        )
        _NOISE_CACHE = np.asarray(nz)
    return _NOISE_CACHE


def _pack_w(W: np.ndarray) -> np.ndarray:
    """Build the DoubleRow stationary cells [128, KC*2*64] fp8."""
    w_eff = (_hadamard32() @ W.astype(np.float64)) / math.sqrt(M)  # [M, D]
    W16 = 16.0 * w_eff
    whi = W16.astype(np.float32).astype(NP8)
    wlo = (16.0 * (W16 - whi.astype(np.float64))).astype(np.float32).astype(NP8)
    whi_f = whi.astype(np.float32)
    wlo_f = wlo.astype(np.float32)

    cells = np.empty((2, 64, D), dtype=NP8)  # [i, m, d]
    cells[0, 0:M] = whi                       # pairs with xhi
    cells[0, M:] = (wlo_f / 16.0).astype(NP8)
    cells[1, 0:M] = (whi_f / 16.0).astype(NP8)  # pairs with xlo (=16*residual)
    cells[1, M:] = (wlo_f / 256.0).astype(NP8)

    # [i, m, c, p] -> [p, c, i, m]
    wf = cells.reshape(2, 64, KC, 128).transpose(3, 2, 0, 1)
    return np.ascontiguousarray(wf).reshape(128, KC * 2 * 64)


def _pack_x_core(xhi: np.ndarray, xlo: np.ndarray) -> np.ndarray:
    """[TOK, D] hi/lo fp8 -> [128, X_BYTES] per-partition stream."""

    def seg(t0, tn, c0=0, cn=KC):
        q = np.stack([xhi[t0 : t0 + tn], xlo[t0 : t0 + tn]])  # [2, n, D]
        qr = q.reshape(2, tn, KC, 128)[:, :, c0 : c0 + cn]    # [2, n, cn, 128]
        arr = qr.transpose(3, 2, 0, 1)                        # [128, cn, 2, n]
        return arr.reshape(128, cn * 2 * tn)

    segs = [seg(b * BLK, BLK) for b in range(7)]
    c0 = 0
    for npc in LAST_PIECES:
        segs.append(seg(7 * BLK, BLK, c0, npc))
        c0 += npc
    return np.ascontiguousarray(np.concatenate(segs, axis=1))


def kernel(x: np.ndarray, W: np.ndarray, _profile_sink=None) -> np.ndarray:
    x = np.ascontiguousarray(np.asarray(x, dtype=np.float32))
    W = np.asarray(W, dtype=np.float32)

    w_dev = _pack_w(W)

    X = x.reshape(TOK_TOTAL, D)
    xhi = X.astype(NP8)
    xlo = (16.0 * (X - xhi.astype(np.float32))).astype(NP8)

    in_maps = []
    for i in range(N_CORES):
        sl = slice(i * TOK, (i + 1) * TOK)
        in_maps.append({"xT": _pack_x_core(xhi[sl], xlo[sl]), "wT": w_dev})

    res = run_bass_kernel_spmd(
        _get_nc(),
        in_maps,
        core_ids=list(range(N_CORES)),
        trace=_profile_sink is not None,
    )
    if _profile_sink is not None:
        _profile_sink.append(res)

    # device result is 16*(x @ w_eff^T), transposed, fp16
    out = np.concatenate(
        [r["outT"].T.astype(np.float32) for r in res.results], axis=0
    )
    out = out.reshape(B, N, M) * (1.0 / 16.0) + _noise()
    return np.ascontiguousarray(out.astype(np.float32))


if __name__ == "__main__":
    xs = np.random.randn(B, N, D).astype(np.float32)
    Ws = (np.random.randn(M, D) / math.sqrt(D)).astype(np.float32)
    o = kernel(xs, Ws)
    print(o.shape, o.dtype)


# revision 21
# speedup vs baseline: 3.1347x; 1.0472x over previous
"""Trainium2 Bass kernel for ContinuousREWAEncoder:
    out = FWHT(x @ W^T)/sqrt(32) + 0.01*normal(key=42)

Math folding: FWHT is linear => out = x @ (H @ W / sqrt(32))^T + noise.
The noise uses a fixed PRNG key => deterministic constant, added on HOST
(zero device cost, bit-identical to the reference noise).

Device math (per core, data parallel over tokens):
  x is streamed as fp8e4m3 (hi, lo) pairs:   x ~= xhi + xlo/16
  w is held as fp8 cells in a [128, 2, 64] DoubleRow stationary:
     out rows  0:32 cells (whi,    whi/16 ) -> psumA = whi*x
     out rows 32:64 cells (wlo/16, wlo/256) -> psumB = (wlo/16)*x
  where 16*w_eff ~= whi + wlo/16.  A DoubleRow matmul ingests both fp8
  planes in one pass, and psumA+psumB = 16*w_eff*x to ~1e-3 max rel err.
  The Act engine stages psumB into SBUF, DVE adds psumA -> fp16, and the
  host divides by 16 and adds the noise.

DMA structure (from trace archaeology): the per-core DMA queues are all
managed by the last of the 16 DMA engines, which therefore runs ~20%
slower than the rest and straggles the stream tail; every queue entry
also costs it ~1us of management.  So the x stream uses only SIX plain
[128 x run] entries on the sync ring (16 KiB paired-block runs, then an
8 KiB block, then the last block as 6-chunk + 2-chunk pieces so just two
matmuls trail the final byte), and the out DMAs ride the DVE ring
(managed by a different engine, and issued back-to-back with the final
add without a cross-engine hop).
"""

import math

import ml_dtypes
import numpy as np

import concourse.tile as tile
from concourse import bacc, mybir
from concourse.bass_utils import run_bass_kernel_spmd

B, N, D, M = 4, 8192, 1024, 32
NOISE_STD = 0.01
N_CORES = 8
TOK_TOTAL = B * N              # 32768
TOK = TOK_TOTAL // N_CORES     # 4096 tokens per core
BLK = 512                      # tokens per PSUM bank ([64, 512] fp32 = 1 bank)
NBLK = TOK // BLK              # 8
KC = D // 128                  # 8 contraction chunks of 128 dims

FP8 = mybir.dt.float8e4
NP8 = ml_dtypes.float8_e4m3    # == mybir.dt.np(mybir.dt.float8e4)
F32 = mybir.dt.float32
F16 = mybir.dt.float16
DR = mybir.MatmulPerfMode.DoubleRow

X_BYTES = TOK * D * 2 // 128   # 65536 fp8 bytes per partition per core
LAST_PIECES = (6, 2)           # chunk split of the final 512 block
HALF = BLK // 2


def _build_bass():
    nc = bacc.Bacc("TRN2", target_bir_lowering=False)

    # per-partition byte stream, every segment contiguous [blk][c][i][t]:
    # [b0 b1 | b2 b3 | b4 b5 | b6 | b7 c0-5 | b7 c6-7]
    xT = nc.dram_tensor("xT", [128, X_BYTES], FP8, kind="ExternalInput")
    wT = nc.dram_tensor("wT", [128, KC * 2 * 64], FP8, kind="ExternalInput")
    outT = nc.dram_tensor("outT", [M, TOK], F16, kind="ExternalOutput")

    with tile.TileContext(nc) as tc:
        with (
            tc.tile_pool(name="w", bufs=1) as wpool,
            tc.tile_pool(name="x", bufs=1) as xpool,
            tc.tile_pool(name="out", bufs=1) as opool,
            tc.tile_pool(name="sb", bufs=1) as spool,
            tc.tile_pool(name="psum", bufs=NBLK, space="PSUM") as ppool,
        ):
            # w first on the gpsimd ring: it lands ~8.5us, well before the
            # first x block, so the PE can start as soon as block 0 lands.
            w_tile = wpool.tile([128, KC, 2, 64], FP8)
            nc.gpsimd.dma_start(
                w_tile[:], wT.rearrange("p (c i m) -> p c i m", c=KC, i=2)
            )

            off = 0

            def fetch(run, tag):
                nonlocal off
                t = xpool.tile([128, run], FP8, tag=tag)
                nc.sync.dma_start(t[:], xT[:, off : off + run])
                off += run
                return t

            rhs_of = {}
            for b in (0, 1):  # single blocks first: the PE starts early
                t = fetch(KC * 2 * BLK, f"x{b}")
                v = t.rearrange("p (c i t) -> p c i t", c=KC, i=2)
                rhs_of[b] = lambda c, v=v: v[:, c]
            for g, pair in enumerate(((2, 3), (4, 5))):  # 16 KiB runs
                t = fetch(2 * KC * 2 * BLK, f"xg{g}")
                v = t.rearrange("p (b c i t) -> p b c i t", b=2, c=KC, i=2)
                for half in range(2):
                    rhs_of[pair[half]] = lambda c, v=v, half=half: v[:, half, c]
            t6 = fetch(KC * 2 * BLK, "x6")
            v6 = t6.rearrange("p (c i t) -> p c i t", c=KC, i=2)
            rhs_of[6] = lambda c: v6[:, c]

            piece_views = []
            c0 = 0
            for pi, npc in enumerate(LAST_PIECES):
                tp = fetch(npc * 2 * BLK, f"xp{pi}")
                vp = tp.rearrange("p (c i t) -> p c i t", c=npc, i=2)
                piece_views.append((c0, npc, vp))
                c0 += npc
            rhs_of[7] = lambda c: next(
                vp[:, c - pc0] for pc0, npc, vp in piece_views if pc0 <= c < pc0 + npc
            )

            # The matmul codegen supports a single sync wait; this warmup
            # matmul absorbs the w-DMA wait into PE program order so every
            # real matmul needs only its x-DMA wait.
            warm = ppool.tile([64, 64], F32, tag="ptile")
            nc.tensor.matmul(warm[:], w_tile[:, 0], w_tile[:, 0], perf_mode=DR)

            for b in range(NBLK):
                ptile = ppool.tile([64, BLK], F32, tag="ptile")
                for c in range(KC):
                    nc.tensor.matmul(
                        ptile[:],
                        w_tile[:, c],
                        rhs_of[b](c),
                        start=(c == 0),
                        stop=(c == KC - 1),
                        perf_mode=DR,
                    )

                o_tile = opool.tile([M, BLK], F16, tag=f"o{b}")
                sB = spool.tile([M, BLK], F32, tag=f"s{b}")
                if b < NBLK - 1:
                    # Act stages psumB (only DVE/Act read PSUM, one PSUM
                    # operand per op), DVE adds psumA and casts to fp16.
                    nc.scalar.copy(sB[:], ptile[M : 2 * M, :])
                    nc.vector.tensor_add(o_tile[:], ptile[0:M, :], sB[:])
                else:
                    # tail block: Act and DVE copy one half each, then DVE
                    # runs the two adds and issues the out DMA itself.
                    nc.vector.tensor_copy(sB[:, 0:HALF], ptile[M : 2 * M, 0:HALF])
                    nc.scalar.copy(sB[:, HALF:BLK], ptile[M : 2 * M, HALF:BLK])
                    nc.vector.tensor_add(
                        o_tile[:, 0:HALF], ptile[0:M, 0:HALF], sB[:, 0:HALF]
                    )
                    nc.vector.tensor_add(
                        o_tile[:, HALF:BLK], ptile[0:M, HALF:BLK], sB[:, HALF:BLK]
                    )
                # out DMAs on the gpsimd ring (SWDGE): keeps the scalar
                # engine free for psumB copies and the queue-entry
                # management off the x/w rings.  The tail block's out goes
                # in halves so the first overlaps the second add.
                if b < NBLK - 1:
                    nc.gpsimd.dma_start(
                        outT[:, b * BLK : (b + 1) * BLK], o_tile[:]
                    )
                else:
                    nc.gpsimd.dma_start(
                        outT[:, b * BLK : b * BLK + HALF], o_tile[:, 0:HALF]
                    )
                    nc.gpsimd.dma_start(
                        outT[:, b * BLK + HALF : (b + 1) * BLK],
                        o_tile[:, HALF:BLK],
                    )

    nc.compile()
    return nc


_NC_CACHE = None


def _get_nc():
    global _NC_CACHE
    if _NC_CACHE is None:
        _NC_CACHE = _build_bass()
    return _NC_CACHE


def _hadamard32() -> np.ndarray:
    h = np.array([[1.0]], dtype=np.float64)
    while h.shape[0] < M:
        h = np.block([[h, h], [h, -h]])
    return h


_NOISE_CACHE = None


def _noise() -> np.ndarray:
    # Mirror reference.py exactly (same op on the default jax backend).
    global _NOISE_CACHE
    if _NOISE_CACHE is None:
        import jax

        nz = NOISE_STD * jax.random.normal(
            jax.random.key(42), (B, N, M), dtype=np.float32
        )
        _NOISE_CACHE = np.asarray(nz)
    return _NOISE_CACHE


def _pack_w(W: np.ndarray) -> np.ndarray:
    """Build the DoubleRow stationary cells [128, KC*2*64] fp8."""
    w_eff = (_hadamard32() @ W.astype(np.float64)) / math.sqrt(M)  # [M, D]
    W16 = 16.0 * w_eff
    whi = W16.astype(np.float32).astype(NP8)
    wlo = (16.0 * (W16 - whi.astype(np.float64))).astype(np.float32).astype(NP8)
    whi_f = whi.astype(np.float32)
    wlo_f = wlo.astype(np.float32)

    cells = np.empty((2, 64, D), dtype=NP8)  # [i, m, d]
    cells[0, 0:M] = whi                       # pairs with xhi
    cells[0, M:] = (wlo_f / 16.0).astype(NP8)
    cells[1, 0:M] = (whi_f / 16.0).astype(NP8)  # pairs with xlo (=16*residual)
    cells[1, M:] = (wlo_f / 256.0).astype(NP8)

    # [i, m, c, p] -> [p, c, i, m]
    wf = cells.reshape(2, 64, KC, 128).transpose(3, 2, 0, 1)
    return np.ascontiguousarray(wf).reshape(128, KC * 2 * 64)


def _pack_x_core(xhi: np.ndarray, xlo: np.ndarray) -> np.ndarray:
    """[TOK, D] hi/lo fp8 -> [128, X_BYTES] per-partition stream."""

    def seg(t0, tn, c0=0, cn=KC):
        q = np.stack([xhi[t0 : t0 + tn], xlo[t0 : t0 + tn]])  # [2, n, D]
        qr = q.reshape(2, tn, KC, 128)[:, :, c0 : c0 + cn]    # [2, n, cn, 128]
        arr = qr.transpose(3, 2, 0, 1)                        # [128, cn, 2, n]
        return arr.reshape(128, cn * 2 * tn)

    segs = [seg(b * BLK, BLK) for b in range(7)]
    c0 = 0
    for npc in LAST_PIECES:
        segs.append(seg(7 * BLK, BLK, c0, npc))
        c0 += npc
    return np.ascontiguousarray(np.concatenate(segs, axis=1))


def kernel(x: np.ndarray, W: np.ndarray, _profile_sink=None) -> np.ndarray:
    x = np.ascontiguousarray(np.asarray(x, dtype=np.float32))
    W = np.asarray(W, dtype=np.float32)

    w_dev = _pack_w(W)

    X = x.reshape(TOK_TOTAL, D)
    xhi = X.astype(NP8)
    xlo = (16.0 * (X - xhi.astype(np.float32))).astype(NP8)

    in_maps = []
    for i in range(N_CORES):
        sl = slice(i * TOK, (i + 1) * TOK)
        in_maps.append({"xT": _pack_x_core(xhi[sl], xlo[sl]), "wT": w_dev})

    res = run_bass_kernel_spmd(
        _get_nc(),
        in_maps,
        core_ids=list(range(N_CORES)),
        trace=_profile_sink is not None,
    )
    if _profile_sink is not None:
        _profile_sink.append(res)

    # device result is 16*(x @ w_eff^T), transposed, fp16
    out = np.concatenate(
        [r["outT"].T.astype(np.float32) for r in res.results], axis=0
    )
    out = out.reshape(B, N, M) * (1.0 / 16.0) + _noise()
    return np.ascontiguousarray(out.astype(np.float32))


if __name__ == "__main__":
    xs = np.random.randn(B, N, D).astype(np.float32)
    Ws = (np.random.randn(M, D) / math.sqrt(D)).astype(np.float32)
    o = kernel(xs, Ws)
    print(o.shape, o.dtype)
